# revision 1
# baseline (speedup 1.0000x reference)
"""Distributed Trainium2 Bass kernel for the phasor attention problem
(nn_Attention_17798344475248).

Sharding: 8 cores = 2 batches x 4 head-groups (2 heads each). Each core
computes its batch's Q/K/V projections for its 2 heads, phasor attention,
and a partial final-dense output; partials are summed with 4 pipelined
4-rank ReduceScatters per batch group; each core finishes atan2 on its
4x64-row slices of the output.

Math notes (vs reference.py):
- phasor_encode(phasor_act(z)) == z/|z|  -> normalize instead of atan2+cos/sin
- softmax max-subtract and sum-normalization cancel in the final angle
  (positive per-row scale), so softmax reduces to exp(s/d)
- complex bias (ones) is folded in as K=1 outer-product matmuls
- f32r (reduced-mantissa, full-speed) is used where errors attenuate
  (Q/K projections, scores); f32 where errors reach the output branch cut
  (V projection, PV, final dense)
- all phasor encodes (the only Sin-set ACT work) run in one phase at the
  start, staged via DRAM, so the ACT table set never thrashes afterwards
"""
import sys

sys.path.insert(0, "/opt/trn_rl_repo")

import numpy as np

import concourse.bass as bass
import concourse.tile as tile
from concourse import bacc, mybir
from concourse.bass_utils import run_bass_kernel_spmd
from concourse.masks import make_identity

F32 = mybir.dt.float32
F32R = mybir.dt.float32r
AF = mybir.ActivationFunctionType
ALU = mybir.AluOpType
PI = float(np.pi)

B, T, D, H = 2, 1024, 512, 8
P = 128
DS = D // P          # 4 partition-slices of the model dim
KVCH = 512           # keyvalue chunk width (t)
QCH = 256            # query chunk width (t)
N_CORES = 8
HPC = 2              # heads per core


def _norm_pair(nc, pools, re_ps, im_ps, re_out, im_out, width):
    """Normalize complex (re,im) [128,width] from PSUM to unit modulus:
    n = exp(-0.5*ln(re^2+im^2)); out = (re*n, im*n)."""
    nt = pools["nt"]
    t1 = nt.tile([P, width], F32, tag="nt")
    nc.scalar.activation(t1[:], re_ps[:], AF.Square, bias=0.0, scale=1.0)
    t2 = nt.tile([P, width], F32, tag="nt")
    nc.scalar.activation(t2[:], im_ps[:], AF.Square, bias=0.0, scale=1.0)
    m = nt.tile([P, width], F32, tag="nt")
    nc.vector.tensor_tensor(m[:], t1[:], t2[:], ALU.add)
    sq = nt.tile([P, width], F32, tag="nt")
    nc.scalar.activation(sq[:], m[:], AF.Sqrt, bias=0.0, scale=1.0)
    n = nt.tile([P, width], F32, tag="nt")
    nc.vector.reciprocal_approx_fast(n[:], sq[:])
    nc.vector.tensor_tensor(re_out, re_ps[:], n[:], ALU.mult)
    nc.vector.tensor_tensor(im_out, im_ps[:], n[:], ALU.mult)


def build(debug=False):
    nc = bacc.Bacc("TRN2", target_bir_lowering=False, debug=False,
                   num_devices=N_CORES)
    cpi2 = nc.alloc_sbuf_tensor("const-f32-pi2", [P, 1], F32)
    nc.gpsimd.memset(cpi2.ap(), PI / 2)
    nc.const_aps.aps[(F32, PI / 2)] = cpi2.ap()
    nc.all_engine_barrier()

    # ---- I/O ----
    QUERY = nc.dram_tensor("query", [T, D], F32, kind="ExternalInput")
    KEYVALUE = nc.dram_tensor("keyvalue", [T, D], F32, kind="ExternalInput")
    WQ = nc.dram_tensor("wq", [HPC, D, D], F32, kind="ExternalInput")
    WK = nc.dram_tensor("wk", [HPC, D, D], F32, kind="ExternalInput")
    WV = nc.dram_tensor("wv", [HPC, D, D], F32, kind="ExternalInput")
    BQ = nc.dram_tensor("bq", [HPC, D], F32, kind="ExternalInput")
    BK = nc.dram_tensor("bk", [HPC, D], F32, kind="ExternalInput")
    BV = nc.dram_tensor("bv", [HPC, D], F32, kind="ExternalInput")
    WO = nc.dram_tensor("wo", [HPC * D, D], F32, kind="ExternalInput")
    BO = nc.dram_tensor("bo", [D], F32, kind="ExternalInput")
    OUT = nc.dram_tensor("out", [T // 4, D], F32, kind="ExternalOutput")

    with tile.TileContext(nc) as tc:
        import contextlib
        with contextlib.ExitStack() as ctx:
            pools = {}
            for name, bufs, space in [
                ("persist", 1, "SBUF"), ("raw", 4, "SBUF"), ("nt", 6, "SBUF"),
                ("w", 1, "SBUF"), ("wr", 4, "SBUF"),
                ("browf", 1, "SBUF"), ("browr", 3, "SBUF"),
                ("kvr", 2, "SBUF"), ("qenc", 2, "SBUF"),
                ("kt", 2, "SBUF"), ("v", 2, "SBUF"),
                ("qt", 2, "SBUF"), ("p", 1, "SBUF"), ("oh", 2, "SBUF"),
                ("z", 4, "SBUF"), ("ps", 8, "PSUM"), ("dram", 1, "DRAM"),
            ]:
                pools[name] = ctx.enter_context(
                    tc.tile_pool(name=name, bufs=bufs, space=space))

            persist = pools["persist"]
            ident = persist.tile([P, P], F32, tag="ident")
            make_identity(nc, ident[:])

            # ---- small constant rows ----
            ones512_f = persist.tile([1, KVCH], F32, tag="ones512f")
            nc.vector.memset(ones512_f[:], 1.0)
            ones512_r = persist.tile([1, KVCH], F32R, tag="ones512r")
            nc.vector.tensor_copy(ones512_r[:], ones512_f[:])
            ones_row_f = ones512_f[0:1, :P]                  # [1,128] f32 = 1.0
            quarter_f = persist.tile([1, P], F32, tag="quarterf")
            nc.vector.memset(quarter_f[:], 0.125)  # bo split over 4 cores x 2 heads
            quarter_r = persist.tile([1, P], F32R, tag="quarterr")
            nc.vector.tensor_copy(quarter_r[:], quarter_f[:])
            bo_r = persist.tile([1, D], F32R, tag="bor")
            bo_f = pools["browf"].tile([1, D], F32, tag="browf", name="bo_f")
            nc.sync.dma_start(bo_f[:], BO[:][None, :])
            nc.vector.tensor_copy(bo_r[:], bo_f[:])

            # ---- DRAM staging ----
            dram = pools["dram"]
            zbs = [dram.tile([KVCH, D], F32, name=f"zb{q}") for q in range(4)]
            rs_outs = [dram.tile([P, D], F32, name=f"rsout{q}") for q in range(4)]
            qe_cos = dram.tile([P, DS, T], F32R, name="qe_cos")
            qe_sin = dram.tile([P, DS, T], F32R, name="qe_sin")
            kve_cos_r = dram.tile([P, DS, T], F32R, name="kve_cos_r")
            kve_sin_r = dram.tile([P, DS, T], F32R, name="kve_sin_r")

            # ================= Phase 1: all phasor encodes =================
            # (the only Sin-set ACT work in the kernel)
            for src_dram, is_q in ((KEYVALUE, False), (QUERY, True)):
                for ch in range(T // KVCH):
                    raw_tiles = []
                    for ts in range(KVCH // P):
                        rt = pools["raw"].tile([P, D], F32, tag="raw")
                        nc.sync.dma_start(
                            rt[:],
                            src_dram[ch * KVCH + ts * P: ch * KVCH + (ts + 1) * P, :])
                        raw_tiles.append(rt)
                    cos_t = pools["kvr"].tile([P, DS, KVCH], F32R, tag="kvr",
                                              name=f"enc_cos_{is_q}_{ch}")
                    sin_t = pools["kvr"].tile([P, DS, KVCH], F32R, tag="kvr",
                                              name=f"enc_sin_{is_q}_{ch}")
                    for ds in range(DS):
                        pt = pools["ps"].tile([P, KVCH], F32, tag="ps")
                        for ts in range(KVCH // P):
                            nc.tensor.transpose(
                                pt[:, ts * P:(ts + 1) * P],
                                raw_tiles[ts][:, ds * P:(ds + 1) * P], ident[:])
                        nc.scalar.activation(sin_t[:, ds, :], pt[:], AF.Sin,
                                             bias=0.0, scale=PI)
                        ab = pools["nt"].tile([P, KVCH], F32, tag="nt")
                        nc.scalar.activation(ab[:], pt[:], AF.Abs, bias=0.0, scale=1.0)
                        nc.scalar.activation(cos_t[:, ds, :], ab[:], AF.Sin,
                                             bias=PI / 2, scale=-PI)
                    chsl = slice(ch * KVCH, (ch + 1) * KVCH)
                    if is_q:
                        nc.sync.dma_start(qe_cos[:, :, chsl], cos_t[:])
                        nc.sync.dma_start(qe_sin[:, :, chsl], sin_t[:])
                    else:
                        nc.sync.dma_start(kve_cos_r[:, :, chsl], cos_t[:])
                        nc.sync.dma_start(kve_sin_r[:, :, chsl], sin_t[:])

            # ================= Phase 2: per-head attention =================
            for h in range(HPC):
                def _bias_row_r(SRC, name):
                    bf = pools["browf"].tile([1, D], F32, tag="browf", name=name + "_f")
                    nc.sync.dma_start(bf[:], SRC[h][None, :])
                    br = pools["browr"].tile([1, D], F32R, tag="browr", name=name + "_r")
                    nc.vector.tensor_copy(br[:], bf[:])
                    return br
                bq_row = _bias_row_r(BQ, f"bq{h}")
                bk_row = _bias_row_r(BK, f"bk{h}")
                bv_row = _bias_row_r(BV, f"bv{h}")

                # ---- weights ----
                wq_r = pools["wr"].tile([P, DS, D], F32R, tag="wr")
                wk_r = pools["wr"].tile([P, DS, D], F32R, tag="wr")
                wv_r = pools["wr"].tile([P, DS, D], F32R, tag="wr")
                wo_r = pools["wr"].tile([P, DS, D], F32R, tag="wr")
                for W_ap, w_r in ((WQ[h], wq_r), (WK[h], wk_r), (WV[h], wv_r),
                                  (WO[h * D:(h + 1) * D, :], wo_r)):
                    wf = pools["w"].tile([P, DS, D], F32, tag="wf")
                    nc.sync.dma_start(
                        wf[:], W_ap.rearrange("(o p) D -> p o D", p=P))
                    nc.vector.tensor_copy(w_r[:], wf[:])

                # ---- persistent per-head tensors ----
                kt_re = pools["kt"].tile([P, DS, T], F32R, tag="kt")
                kt_im = pools["kt"].tile([P, DS, T], F32R, tag="kt")
                v_re = pools["v"].tile([P, T // P, D], F32R, tag="v")
                v_im = pools["v"].tile([P, T // P, D], F32R, tag="v")

                # ======== KV pass: K^T (f32r) and V (f32) ========
                for ch in range(T // KVCH):
                    chsl = slice(ch * KVCH, (ch + 1) * KVCH)
                    kv_cos_r = pools["kvr"].tile([P, DS, KVCH], F32R, tag="kvr",
                                                 name=f"kvcr_{h}_{ch}")
                    kv_sin_r = pools["kvr"].tile([P, DS, KVCH], F32R, tag="kvr",
                                                 name=f"kvsr_{h}_{ch}")
                    nc.sync.dma_start(kv_cos_r[:], kve_cos_r[:, :, chsl])
                    nc.sync.dma_start(kv_sin_r[:], kve_sin_r[:, :, chsl])

                    # V projection (f32)
                    for ts in range(KVCH // P):
                        pre = pools["ps"].tile([P, D], F32, tag="ps")
                        pim = pools["ps"].tile([P, D], F32, tag="ps")
                        for do in range(DS):
                            nc.tensor.matmul(
                                pre[:], lhsT=kv_cos_r[:, do, ts * P:(ts + 1) * P],
                                rhs=wv_r[:, do, :], start=(do == 0), stop=False)
                        nc.tensor.matmul(
                            pre[:], lhsT=ones512_r[0:1, :P], rhs=bv_row[:],
                            start=False, stop=True)
                        for do in range(DS):
                            nc.tensor.matmul(
                                pim[:], lhsT=kv_sin_r[:, do, ts * P:(ts + 1) * P],
                                rhs=wv_r[:, do, :], start=(do == 0),
                                stop=(do == DS - 1))
                        trow = ch * (KVCH // P) + ts
                        _norm_pair(nc, pools, pre, pim,
                                   v_re[:, trow, :], v_im[:, trow, :], D)

                    # K projection (f32r): K^T [D', t]
                    for dso in range(DS):
                        pre = pools["ps"].tile([P, KVCH], F32, tag="ps")
                        pim = pools["ps"].tile([P, KVCH], F32, tag="ps")
                        for do in range(DS):
                            nc.tensor.matmul(
                                pre[:], lhsT=wk_r[:, do, dso * P:(dso + 1) * P],
                                rhs=kv_cos_r[:, do, :], start=(do == 0), stop=False)
                        nc.tensor.matmul(
                            pre[:], lhsT=bk_row[0:1, dso * P:(dso + 1) * P],
                            rhs=ones512_r[:], start=False, stop=True)
                        for do in range(DS):
                            nc.tensor.matmul(
                                pim[:], lhsT=wk_r[:, do, dso * P:(dso + 1) * P],
                                rhs=kv_sin_r[:, do, :], start=(do == 0),
                                stop=(do == DS - 1))
                        _norm_pair(nc, pools, pre, pim,
                                   kt_re[:, dso, chsl], kt_im[:, dso, chsl], KVCH)

                # ======== Q pass per 256-chunk ========
                for ch in range(T // QCH):
                    qsl = slice(ch * QCH, (ch + 1) * QCH)
                    q_cos_r = pools["qenc"].tile([P, DS, QCH], F32R, tag="qenc",
                                                 name=f"qc_{h}_{ch}")
                    q_sin_r = pools["qenc"].tile([P, DS, QCH], F32R, tag="qenc",
                                                 name=f"qs_{h}_{ch}")
                    nc.sync.dma_start(q_cos_r[:], qe_cos[:, :, qsl])
                    nc.sync.dma_start(q_sin_r[:], qe_sin[:, :, qsl])

                    qt_re = pools["qt"].tile([P, DS, QCH], F32R, tag="qt")
                    qt_im = pools["qt"].tile([P, DS, QCH], F32R, tag="qt")
                    for dso in range(DS):
                        pre = pools["ps"].tile([P, QCH], F32, tag="ps")
                        pim = pools["ps"].tile([P, QCH], F32, tag="ps")
                        for do in range(DS):
                            nc.tensor.matmul(
                                pre[:], lhsT=wq_r[:, do, dso * P:(dso + 1) * P],
                                rhs=q_cos_r[:, do, :], start=(do == 0), stop=False)
                        nc.tensor.matmul(
                            pre[:], lhsT=bq_row[0:1, dso * P:(dso + 1) * P],
                            rhs=ones512_r[0:1, :QCH], start=False, stop=True)
                        for do in range(DS):
                            nc.tensor.matmul(
                                pim[:], lhsT=wq_r[:, do, dso * P:(dso + 1) * P],
                                rhs=q_sin_r[:, do, :], start=(do == 0),
                                stop=(do == DS - 1))
                        _norm_pair(nc, pools, pre, pim,
                                   qt_re[:, dso, :], qt_im[:, dso, :], QCH)

                    # scores + exp -> P^T [Tkv, tq-chunk]
                    pt_all = pools["p"].tile([P, T // P, QCH], F32R, tag="p")
                    for to in range(T // P):
                        ps_s = pools["ps"].tile([P, QCH], F32, tag="ps")
                        for do in range(DS):
                            nc.tensor.matmul(
                                ps_s[:], lhsT=kt_re[:, do, to * P:(to + 1) * P],
                                rhs=qt_re[:, do, :], start=(do == 0), stop=False)
                        for do in range(DS):
                            nc.tensor.matmul(
                                ps_s[:], lhsT=kt_im[:, do, to * P:(to + 1) * P],
                                rhs=qt_im[:, do, :], start=False,
                                stop=(do == DS - 1))
                        nc.scalar.activation(pt_all[:, to, :], ps_s[:], AF.Exp,
                                             bias=0.0, scale=1.0 / D)

                    # PV (f32): O^T [D', tq-chunk], two groups of 2 D'-slices
                    oh_re = pools["oh"].tile([P, DS, QCH], F32R, tag="oh")
                    oh_im = pools["oh"].tile([P, DS, QCH], F32R, tag="oh")
                    for grp in range(2):
                        ps_tiles = {}
                        for dso in (2 * grp, 2 * grp + 1):
                            ps_tiles[(dso, 0)] = pools["ps"].tile(
                                [P, QCH], F32, tag="ps", name=f"pv_{h}_{ch}_{dso}_re")
                            ps_tiles[(dso, 1)] = pools["ps"].tile(
                                [P, QCH], F32, tag="ps", name=f"pv_{h}_{ch}_{dso}_im")
                        for to in range(T // P):
                            for dso in (2 * grp, 2 * grp + 1):
                                nc.tensor.matmul(
                                    ps_tiles[(dso, 0)][:],
                                    lhsT=v_re[:, to, dso * P:(dso + 1) * P],
                                    rhs=pt_all[:, to, :], start=(to == 0),
                                    stop=(to == T // P - 1))
                                nc.tensor.matmul(
                                    ps_tiles[(dso, 1)][:],
                                    lhsT=v_im[:, to, dso * P:(dso + 1) * P],
                                    rhs=pt_all[:, to, :], start=(to == 0),
                                    stop=(to == T // P - 1))
                        for dso in (2 * grp, 2 * grp + 1):
                            _norm_pair(nc, pools, ps_tiles[(dso, 0)],
                                       ps_tiles[(dso, 1)],
                                       oh_re[:, dso, :], oh_im[:, dso, :], QCH)

                    # final dense partial: Z [tq, D'] += Ohat^T.T @ wo_h
                    for ts in range(QCH // P):
                        pzre = pools["ps"].tile([P, D], F32, tag="ps")
                        pzim = pools["ps"].tile([P, D], F32, tag="ps")
                        for do in range(DS):
                            nc.tensor.matmul(
                                pzre[:], lhsT=oh_re[:, do, ts * P:(ts + 1) * P],
                                rhs=wo_r[:, do, :], start=(do == 0), stop=False)
                        nc.tensor.matmul(
                            pzre[:], lhsT=quarter_r[:], rhs=bo_r[:],
                            start=False, stop=True)
                        for do in range(DS):
                            nc.tensor.matmul(
                                pzim[:], lhsT=oh_im[:, do, ts * P:(ts + 1) * P],
                                rhs=wo_r[:, do, :], start=(do == 0),
                                stop=(do == DS - 1))
                        # zb row layout (per tq quarter qq of 256 rows):
                        #   qq*512 + r*128 + c*64 + i, r = rank-block, c = 0 re / 1 im
                        tq0 = ch * QCH + ts * P
                        qq = tq0 // 256
                        u0 = tq0 % 256        # 0 or 128 within the quarter
                        r0 = u0 // 64
                        zre_sb = pools["z"].tile([P, D], F32, tag="z")
                        zim_sb = pools["z"].tile([P, D], F32, tag="z")
                        if h == 0:
                            nc.vector.tensor_copy(zre_sb[:], pzre[:])
                            nc.vector.tensor_copy(zim_sb[:], pzim[:])
                        else:
                            # fuse head-0 partial add into the PSUM drain
                            h0re = pools["z"].tile([P, D], F32, tag="z",
                                                   name=f"h0re_{ch}_{ts}")
                            h0im = pools["z"].tile([P, D], F32, tag="z",
                                                   name=f"h0im_{ch}_{ts}")
                            for half in range(2):
                                r_ = r0 + half
                                src = slice(half * 64, (half + 1) * 64)
                                nc.sync.dma_start(h0re[src, :],
                                                  zbs[qq][r_ * P: r_ * P + 64, :])
                                nc.sync.dma_start(h0im[src, :],
                                                  zbs[qq][r_ * P + 64: r_ * P + 128, :])
                            nc.vector.tensor_tensor(zre_sb[:], pzre[:], h0re[:],
                                                    ALU.add)
                            nc.vector.tensor_tensor(zim_sb[:], pzim[:], h0im[:],
                                                    ALU.add)
                        for half in range(2):  # two 64-row rank blocks per tile
                            r_ = r0 + half
                            dst_re = zbs[qq][r_ * P: r_ * P + 64, :]
                            dst_im = zbs[qq][r_ * P + 64: r_ * P + 128, :]
                            src = slice(half * 64, (half + 1) * 64)
                            nc.sync.dma_start(dst_re, zre_sb[src, :])
                            nc.sync.dma_start(dst_im, zim_sb[src, :])

                    # fire the sub-ReduceScatter for this tq quarter once the
                    # second head's accumulation for it has landed
                    if h == HPC - 1:
                        qq = ch  # QCH == 256 -> chunk == quarter
                        nc.gpsimd.collective_compute(
                            "ReduceScatter", ALU.add,
                            replica_groups=[[0, 1, 2, 3], [4, 5, 6, 7]],
                            ins=[zbs[qq].opt()],
                            outs=[rs_outs[qq].opt()],
                        )

            # ======== atan2(zim, zre)/pi, quarters batched in pairs ========
            for pp in range(2):
                qa, qb = 2 * pp, 2 * pp + 1
                zre_t = pools["z"].tile([P, D], F32, tag="z", name=f"zre{pp}")
                nc.sync.dma_start(zre_t[0:64, :], rs_outs[qa][0:64, :])
                nc.sync.dma_start(zre_t[64:128, :], rs_outs[qb][0:64, :])
                zim_t = pools["z"].tile([P, D], F32, tag="z", name=f"zim{pp}")
                nc.sync.dma_start(zim_t[0:64, :], rs_outs[qa][64:128, :])
                nc.sync.dma_start(zim_t[64:128, :], rs_outs[qb][64:128, :])
                zre, zim = zre_t[:, :], zim_t[:, :]
                nt = pools["nt"]

                def ft(nm, pp=pp):
                    return nt.tile([P, D], F32, tag="nt", name=f"{nm}{pp}")
                t1 = ft("f1")
                nc.scalar.activation(t1[:], zre, AF.Square, bias=0.0, scale=1.0)
                t2 = ft("f2")
                nc.vector.tensor_tensor(t2[:], zim, zim, ALU.mult)
                m = ft("f3")
                nc.vector.tensor_tensor(m[:], t1[:], t2[:], ALU.add)
                az = ft("f5")
                nc.scalar.activation(az[:], m[:], AF.Sqrt, bias=0.0, scale=1.0)
                den1 = ft("f6")
                nc.vector.tensor_tensor(den1[:], az[:], zre, ALU.add)
                r1 = ft("f7")
                nc.vector.reciprocal_approx_fast(r1[:], den1[:])
                ta0 = ft("f8")
                nc.vector.tensor_tensor(ta0[:], zim, r1[:], ALU.mult)
                ta = ft("f9")
                nc.vector.tensor_scalar(ta[:], ta0[:], 1e8, -1e8, ALU.min, ALU.max)
                num2 = ft("fa")
                nc.vector.tensor_tensor(num2[:], az[:], zre, ALU.subtract)
                r2 = ft("fb")
                nc.vector.reciprocal_approx_fast(r2[:], zim)
                tb0 = ft("fc")
                nc.vector.tensor_tensor(tb0[:], num2[:], r2[:], ALU.mult)
                tb = ft("fd")
                nc.vector.tensor_scalar(tb[:], tb0[:], 1e8, -1e8, ALU.min, ALU.max)
                ata = ft("fe")
                nc.scalar.activation(ata[:], ta[:], AF.Arctan, bias=0.0, scale=1.0)
                atb = ft("ff")
                nc.scalar.activation(atb[:], tb[:], AF.Arctan, bias=0.0, scale=1.0)
                mask = ft("fg")
                nc.vector.tensor_scalar(mask[:], zre, 0.0, None, ALU.is_ge)
                dsel = ft("fh")
                nc.vector.tensor_tensor(dsel[:], ata[:], atb[:], ALU.subtract)
                md = ft("fi")
                nc.vector.tensor_tensor(md[:], mask[:], dsel[:], ALU.mult)
                sel = ft("fj")
                nc.vector.tensor_tensor(sel[:], atb[:], md[:], ALU.add)
                outt = ft("fk")
                nc.vector.tensor_scalar(outt[:], sel[:], 2.0 / PI, None, ALU.mult)
                nc.sync.dma_start(OUT[qa * 64:(qa + 1) * 64, :], outt[0:64, :])
                nc.sync.dma_start(OUT[qb * 64:(qb + 1) * 64, :], outt[64:128, :])

    nc.finalize()
    return nc


_NC_CACHE = {}


def _get_nc():
    if "nc" not in _NC_CACHE:
        _NC_CACHE["nc"] = build()
    return _NC_CACHE["nc"]


def kernel(**inputs):
    query = np.ascontiguousarray(np.asarray(inputs["query"], dtype=np.float32))
    keyvalue = np.ascontiguousarray(np.asarray(inputs["keyvalue"], dtype=np.float32))
    wq = np.asarray(inputs["wq"], dtype=np.float32)
    wk = np.asarray(inputs["wk"], dtype=np.float32)
    wv = np.asarray(inputs["wv"], dtype=np.float32)
    bq = np.asarray(inputs["bq"], dtype=np.float32)
    bk = np.asarray(inputs["bk"], dtype=np.float32)
    bv = np.asarray(inputs["bv"], dtype=np.float32)
    wo = np.asarray(inputs["wo"], dtype=np.float32)
    bo = np.asarray(inputs["bo"], dtype=np.float32)

    in_maps = []
    for c in range(N_CORES):
        b, g = c // 4, c % 4
        h0 = g * HPC
        in_maps.append({
            "query": query[b],
            "keyvalue": keyvalue[b],
            "wq": np.ascontiguousarray(wq[h0:h0 + HPC]),
            "wk": np.ascontiguousarray(wk[h0:h0 + HPC]),
            "wv": np.ascontiguousarray(wv[h0:h0 + HPC]),
            "bq": np.ascontiguousarray(bq[h0:h0 + HPC]),
            "bk": np.ascontiguousarray(bk[h0:h0 + HPC]),
            "bv": np.ascontiguousarray(bv[h0:h0 + HPC]),
            "wo": np.ascontiguousarray(wo[h0 * D:(h0 + HPC) * D]),
            "bo": bo,
        })

    nc = _get_nc()
    res = run_bass_kernel_spmd(nc, in_maps, core_ids=list(range(N_CORES)))
    _NC_CACHE["last_results"] = res
    out = np.empty((B, T, D), np.float32)
    for c in range(N_CORES):
        b, g = c // 4, c % 4
        o = res.results[c]["out"]          # [256, 512]: 4 quarters x 64 rows
        for qq in range(4):
            out[b, qq * 256 + g * 64: qq * 256 + (g + 1) * 64, :] = \
                o[qq * 64:(qq + 1) * 64, :]
    return out



# revision 11
# speedup vs baseline: 1.2653x; 1.2653x over previous
"""Distributed Trainium2 Bass kernel for the phasor attention problem
(nn_Attention_17798344475248).

Sharding: 8 cores = 2 batches x 4 head-groups (2 heads each). Each core
computes its batch's Q/K/V projections for its 2 heads, phasor attention,
and a partial final-dense output; partials are summed with 4 pipelined
4-rank ReduceScatters per batch group; each core finishes atan2 on its
4x64-row slices of the output.

Math notes (vs reference.py):
- phasor_encode(phasor_act(z)) == z/|z|  -> normalize instead of atan2+cos/sin
- softmax max-subtract and sum-normalization cancel in the final angle
  (positive per-row scale), so softmax reduces to exp(s/d)
- complex bias (real) folds into the ACT Square/rescale passes for Q/K
  (per-partition bias) and stays a K=1 outer-product matmul for V / final
- all matmul operands are fp16 (10-bit mantissa, 1 PE cycle/row, fast
  weight load) with f32 PSUM accumulation
- phasor encodes (the only Sin-set ACT work) run in one pipelined phase
  at the start and stay resident in SBUF as fp16
"""
import sys

sys.path.insert(0, "/opt/trn_rl_repo")

import numpy as np

import concourse.bass as bass
import concourse.tile as tile
from concourse import bacc, mybir
from concourse.bass_utils import run_bass_kernel_spmd
from concourse.masks import make_identity

F32 = mybir.dt.float32
F16 = mybir.dt.float16
AF = mybir.ActivationFunctionType
ALU = mybir.AluOpType
PI = float(np.pi)

B, T, D, H = 2, 1024, 512, 8
P = 128
DS = D // P          # 4 partition-slices of the model dim
CH = 512             # chunk width along t (both q and kv passes)
NCH = T // CH        # 2 chunks
N_CORES = 8
HPC = 2              # heads per core


def build(debug=False):
    nc = bacc.Bacc("TRN2", target_bir_lowering=False, debug=False,
                   num_devices=N_CORES)
    cpi2 = nc.alloc_sbuf_tensor("const-f32-pi2", [P, 1], F32)
    nc.gpsimd.memset(cpi2.ap(), PI / 2)
    nc.const_aps.aps[(F32, PI / 2)] = cpi2.ap()
    nc.all_engine_barrier()

    # ---- I/O ----
    QUERY = nc.dram_tensor("query", [T, D], F32, kind="ExternalInput")
    KEYVALUE = nc.dram_tensor("keyvalue", [T, D], F32, kind="ExternalInput")
    WQ = nc.dram_tensor("wq", [HPC, D, D], F32, kind="ExternalInput")
    WK = nc.dram_tensor("wk", [HPC, D, D], F32, kind="ExternalInput")
    WV = nc.dram_tensor("wv", [HPC, D, D], F32, kind="ExternalInput")
    BQ = nc.dram_tensor("bq", [HPC, D], F32, kind="ExternalInput")
    BK = nc.dram_tensor("bk", [HPC, D], F32, kind="ExternalInput")
    BV = nc.dram_tensor("bv", [HPC, D], F32, kind="ExternalInput")
    WO = nc.dram_tensor("wo", [HPC * D, D], F32, kind="ExternalInput")
    BO = nc.dram_tensor("bo", [D], F32, kind="ExternalInput")
    OUT = nc.dram_tensor("out", [T // 4, D], F32, kind="ExternalOutput")

    with tile.TileContext(nc) as tc:
        import contextlib
        with contextlib.ExitStack() as ctx:
            pools = {}
            for name, bufs, space in [
                ("persist", 1, "SBUF"),
                ("raw", 4, "SBUF"),       # 2KB x4 raw input tiles
                ("nt", 7, "SBUF"),        # 2KB x7 norm/atan2 temps
                ("w", 1, "SBUF"),         # 8KB f32 weight staging
                ("w16", 4, "SBUF"),       # 2KB x4 fp16 weights (per head)
                ("brow", 6, "SBUF"),      # small bias rows/cols
                ("enc", 1, "SBUF"),       # 32KB: q/kv cos/sin fp16 (4 tags)
                ("kt", 2, "SBUF"),        # 16KB: K^T fp16 (re+im)
                ("v", 2, "SBUF"),         # 16KB: V fp16 (re+im)
                ("qt", 4, "SBUF"),        # 8KB: Q^T fp16 (re+im, 2 chunks)
                ("p", 2, "SBUF"),         # 8KB x2: probs fp16 per chunk
                ("oh", 4, "SBUF"),        # 16KB: PV out fp16 (re+im, 2 chunks)
                ("z", 1, "SBUF"),         # 32KB: f32 z accumulators (2 tags)
                ("ps", 8, "PSUM"),
                ("dram", 1, "DRAM"),
            ]:
                pools[name] = ctx.enter_context(
                    tc.tile_pool(name=name, bufs=bufs, space=space))

            persist = pools["persist"]
            ident = persist.tile([P, P], F32, tag="ident")
            make_identity(nc, ident[:])

            # ---- small constants ----
            ones_f = persist.tile([1, P], F32, tag="onesf")
            nc.vector.memset(ones_f[:], 1.0)
            ones16 = persist.tile([1, P], F16, tag="ones16")
            nc.vector.tensor_copy(ones16[:], ones_f[:])
            quart_f = persist.tile([1, P], F32, tag="quartf")
            nc.vector.memset(quart_f[:], 0.25)   # bo split over 4 cores
            quart16 = persist.tile([1, P], F16, tag="quart16")
            nc.vector.tensor_copy(quart16[:], quart_f[:])
            bo_f = pools["brow"].tile([1, D], F32, tag="brow", name="bo_f")
            nc.sync.dma_start(bo_f[:], BO[:][None, :])
            bo16 = persist.tile([1, D], F16, tag="bo16")
            nc.vector.tensor_copy(bo16[:], bo_f[:])

            # ---- DRAM staging for the collective ----
            dram = pools["dram"]
            zbs = [dram.tile([CH, D], F32, name=f"zb{q}") for q in range(4)]
            rs_outs = [dram.tile([P, D], F32, name=f"rsout{q}") for q in range(4)]

            # ---- persistent fp16 encodes:  [128, DS, T] (D' x t layout) ----
            enc = pools["enc"]
            q_cos = enc.tile([P, DS, T], F16, tag="qc", name="q_cos")
            q_sin = enc.tile([P, DS, T], F16, tag="qs", name="q_sin")
            kv_cos = enc.tile([P, DS, T], F16, tag="kvc", name="kv_cos")
            kv_sin = enc.tile([P, DS, T], F16, tag="kvs", name="kv_sin")

            # ================= Phase 1: phasor encodes =================
            # (the only Sin-set ACT work in the kernel)
            for src_dram, cos_t, sin_t in ((KEYVALUE, kv_cos, kv_sin),
                                           (QUERY, q_cos, q_sin)):
                for ch in range(NCH):
                    chsl = slice(ch * CH, (ch + 1) * CH)
                    raw_tiles = []
                    for ts in range(CH // P):
                        rt = pools["raw"].tile([P, D], F32, tag="raw")
                        nc.sync.dma_start(
                            rt[:],
                            src_dram[ch * CH + ts * P: ch * CH + (ts + 1) * P, :])
                        raw_tiles.append(rt)
                    for ds in range(DS):
                        pt = pools["ps"].tile([P, CH], F32, tag="ps")
                        for ts in range(CH // P):
                            nc.tensor.transpose(
                                pt[:, ts * P:(ts + 1) * P],
                                raw_tiles[ts][:, ds * P:(ds + 1) * P], ident[:])
                        nc.scalar.activation(sin_t[:, ds, chsl], pt[:], AF.Sin,
                                             bias=0.0, scale=PI)
                        ab = pools["nt"].tile([P, CH], F32, tag="nt")
                        nc.scalar.activation(ab[:], pt[:], AF.Abs,
                                             bias=0.0, scale=1.0)
                        nc.scalar.activation(cos_t[:, ds, chsl], ab[:], AF.Sin,
                                             bias=PI / 2, scale=-PI)

            # ---- f32 z accumulators (summed across the 2 heads) ----
            z_re = pools["z"].tile([P, T // P, D], F32, tag="zre", name="z_re")
            z_im = pools["z"].tile([P, T // P, D], F32, tag="zim", name="z_im")

            # ================= Phase 2: per-head attention =================
            for h in range(HPC):
                # ---- biases ----
                # bq/bk as [128, DS] per-partition columns (for ACT folding)
                bq_col = pools["brow"].tile([P, DS], F32, tag="brow",
                                            name=f"bqc{h}")
                nc.sync.dma_start(bq_col[:],
                                  BQ[h].rearrange("(o p) -> p o", p=P))
                bk_col = pools["brow"].tile([P, DS], F32, tag="brow",
                                            name=f"bkc{h}")
                nc.sync.dma_start(bk_col[:],
                                  BK[h].rearrange("(o p) -> p o", p=P))
                bv_f = pools["brow"].tile([1, D], F32, tag="brow",
                                          name=f"bvr{h}")
                nc.sync.dma_start(bv_f[:], BV[h][None, :])
                bv16 = pools["brow"].tile([1, D], F16, tag="brow",
                                          name=f"bvr16{h}")
                nc.vector.tensor_copy(bv16[:], bv_f[:])

                # ---- weights -> fp16 ----
                wq16 = pools["w16"].tile([P, DS, D], F16, tag="w16",
                                         name=f"wq16_{h}")
                wk16 = pools["w16"].tile([P, DS, D], F16, tag="w16",
                                         name=f"wk16_{h}")
                wv16 = pools["w16"].tile([P, DS, D], F16, tag="w16",
                                         name=f"wv16_{h}")
                wo16 = pools["w16"].tile([P, DS, D], F16, tag="w16",
                                         name=f"wo16_{h}")
                for W_ap, w16 in ((WQ[h], wq16), (WK[h], wk16), (WV[h], wv16),
                                  (WO[h * D:(h + 1) * D, :], wo16)):
                    wf = pools["w"].tile([P, DS, D], F32, tag="wf")
                    nc.sync.dma_start(
                        wf[:], W_ap.rearrange("(o p) D -> p o D", p=P))
                    nc.vector.tensor_copy(w16[:], wf[:])

                # ---- persistent per-head tensors ----
                kt_re = pools["kt"].tile([P, DS, T], F16, tag="kt")
                kt_im = pools["kt"].tile([P, DS, T], F16, tag="kt")
                v_re = pools["v"].tile([P, T // P, D], F16, tag="v")
                v_im = pools["v"].tile([P, T // P, D], F16, tag="v")

                # ======== KV pass: K^T and V ========
                for ch in range(NCH):
                    chsl = slice(ch * CH, (ch + 1) * CH)

                    # K projection: K^T [D', t], bias folded via ACT
                    for dso in range(DS):
                        pre = pools["ps"].tile([P, CH], F32, tag="ps")
                        pim = pools["ps"].tile([P, CH], F32, tag="ps")
                        for do in range(DS):
                            nc.tensor.matmul(
                                pre[:], lhsT=wk16[:, do, dso * P:(dso + 1) * P],
                                rhs=kv_cos[:, do, chsl], start=(do == 0),
                                stop=(do == DS - 1))
                            nc.tensor.matmul(
                                pim[:], lhsT=wk16[:, do, dso * P:(dso + 1) * P],
                                rhs=kv_sin[:, do, chsl], start=(do == 0),
                                stop=(do == DS - 1))
                        b_ap = bk_col[:, dso:dso + 1]
                        nt = pools["nt"]
                        t1 = nt.tile([P, CH], F32, tag="nt")
                        nc.scalar.activation(t1[:], pre[:], AF.Square,
                                             bias=b_ap, scale=1.0)
                        t2 = nt.tile([P, CH], F32, tag="nt")
                        nc.scalar.activation(t2[:], pim[:], AF.Square,
                                             bias=0.0, scale=1.0)
                        m = nt.tile([P, CH], F32, tag="nt")
                        nc.vector.tensor_tensor(m[:], t1[:], t2[:], ALU.add)
                        n = nt.tile([P, CH], F32, tag="nt")
                        nc.scalar.activation(n[:], m[:], AF.Abs_reciprocal_sqrt,
                                             bias=0.0, scale=1.0)
                        nc.vector.scalar_tensor_tensor(
                            kt_re[:, dso, chsl], pre[:], b_ap, n[:],
                            ALU.add, ALU.mult)
                        nc.vector.tensor_tensor(kt_im[:, dso, chsl], pim[:],
                                                n[:], ALU.mult)

                    # V projection: V [t, D], bias as K=1 matmul
                    for tb in range(CH // P):
                        tsl = slice(ch * CH + tb * P, ch * CH + (tb + 1) * P)
                        pre = pools["ps"].tile([P, D], F32, tag="ps")
                        pim = pools["ps"].tile([P, D], F32, tag="ps")
                        for do in range(DS):
                            nc.tensor.matmul(
                                pre[:], lhsT=kv_cos[:, do, tsl],
                                rhs=wv16[:, do, :], start=(do == 0), stop=False)
                            nc.tensor.matmul(
                                pim[:], lhsT=kv_sin[:, do, tsl],
                                rhs=wv16[:, do, :], start=(do == 0),
                                stop=(do == DS - 1))
                        nc.tensor.matmul(
                            pre[:], lhsT=ones16[:], rhs=bv16[:],
                            start=False, stop=True)
                        trow = ch * (CH // P) + tb
                        nt = pools["nt"]
                        t1 = nt.tile([P, D], F32, tag="nt")
                        nc.scalar.activation(t1[:], pre[:], AF.Square,
                                             bias=0.0, scale=1.0)
                        t2 = nt.tile([P, D], F32, tag="nt")
                        nc.scalar.activation(t2[:], pim[:], AF.Square,
                                             bias=0.0, scale=1.0)
                        m = nt.tile([P, D], F32, tag="nt")
                        nc.vector.tensor_tensor(m[:], t1[:], t2[:], ALU.add)
                        n = nt.tile([P, D], F32, tag="nt")
                        nc.scalar.activation(n[:], m[:], AF.Abs_reciprocal_sqrt,
                                             bias=0.0, scale=1.0)
                        nc.vector.tensor_tensor(v_re[:, trow, :], pre[:], n[:],
                                                ALU.mult)
                        nc.vector.tensor_tensor(v_im[:, trow, :], pim[:], n[:],
                                                ALU.mult)

                # ======== Q pass per 512-chunk ========
                for ch in range(NCH):
                    qsl = slice(ch * CH, (ch + 1) * CH)

                    # Q projection: Q^T [D', t], bias folded via ACT
                    qt_re = pools["qt"].tile([P, DS, CH], F16, tag="qt")
                    qt_im = pools["qt"].tile([P, DS, CH], F16, tag="qt")
                    for dso in range(DS):
                        pre = pools["ps"].tile([P, CH], F32, tag="ps")
                        pim = pools["ps"].tile([P, CH], F32, tag="ps")
                        for do in range(DS):
                            nc.tensor.matmul(
                                pre[:], lhsT=wq16[:, do, dso * P:(dso + 1) * P],
                                rhs=q_cos[:, do, qsl], start=(do == 0),
                                stop=(do == DS - 1))
                            nc.tensor.matmul(
                                pim[:], lhsT=wq16[:, do, dso * P:(dso + 1) * P],
                                rhs=q_sin[:, do, qsl], start=(do == 0),
                                stop=(do == DS - 1))
                        b_ap = bq_col[:, dso:dso + 1]
                        nt = pools["nt"]
                        t1 = nt.tile([P, CH], F32, tag="nt")
                        nc.scalar.activation(t1[:], pre[:], AF.Square,
                                             bias=b_ap, scale=1.0)
                        t2 = nt.tile([P, CH], F32, tag="nt")
                        nc.scalar.activation(t2[:], pim[:], AF.Square,
                                             bias=0.0, scale=1.0)
                        m = nt.tile([P, CH], F32, tag="nt")
                        nc.vector.tensor_tensor(m[:], t1[:], t2[:], ALU.add)
                        n = nt.tile([P, CH], F32, tag="nt")
                        nc.scalar.activation(n[:], m[:], AF.Abs_reciprocal_sqrt,
                                             bias=0.0, scale=1.0)
                        nc.vector.scalar_tensor_tensor(
                            qt_re[:, dso, :], pre[:], b_ap, n[:],
                            ALU.add, ALU.mult)
                        nc.vector.tensor_tensor(qt_im[:, dso, :], pim[:],
                                                n[:], ALU.mult)

                    # scores + exp -> P^T [Tkv, tq-chunk] fp16
                    pt_all = pools["p"].tile([P, T // P, CH], F16, tag="p")
                    for to in range(T // P):
                        ps_s = pools["ps"].tile([P, CH], F32, tag="ps")
                        for do in range(DS):
                            nc.tensor.matmul(
                                ps_s[:], lhsT=kt_re[:, do, to * P:(to + 1) * P],
                                rhs=qt_re[:, do, :], start=(do == 0), stop=False)
                        for do in range(DS):
                            nc.tensor.matmul(
                                ps_s[:], lhsT=kt_im[:, do, to * P:(to + 1) * P],
                                rhs=qt_im[:, do, :], start=False,
                                stop=(do == DS - 1))
                        nc.scalar.activation(pt_all[:, to, :], ps_s[:], AF.Exp,
                                             bias=0.0, scale=1.0 / D)

                    # PV: O^T [D', tq-chunk], two groups of 2 D'-slices
                    oh_re = pools["oh"].tile([P, DS, CH], F16, tag="oh")
                    oh_im = pools["oh"].tile([P, DS, CH], F16, tag="oh")
                    for grp in range(2):
                        ps_tiles = {}
                        for dso in (2 * grp, 2 * grp + 1):
                            ps_tiles[(dso, 0)] = pools["ps"].tile(
                                [P, CH], F32, tag="ps", name=f"pv_{h}_{ch}_{dso}_re")
                            ps_tiles[(dso, 1)] = pools["ps"].tile(
                                [P, CH], F32, tag="ps", name=f"pv_{h}_{ch}_{dso}_im")
                        for to in range(T // P):
                            for dso in (2 * grp, 2 * grp + 1):
                                nc.tensor.matmul(
                                    ps_tiles[(dso, 0)][:],
                                    lhsT=v_re[:, to, dso * P:(dso + 1) * P],
                                    rhs=pt_all[:, to, :], start=(to == 0),
                                    stop=(to == T // P - 1))
                                nc.tensor.matmul(
                                    ps_tiles[(dso, 1)][:],
                                    lhsT=v_im[:, to, dso * P:(dso + 1) * P],
                                    rhs=pt_all[:, to, :], start=(to == 0),
                                    stop=(to == T // P - 1))
                        for dso in (2 * grp, 2 * grp + 1):
                            pre, pim = ps_tiles[(dso, 0)], ps_tiles[(dso, 1)]
                            nt = pools["nt"]
                            t1 = nt.tile([P, CH], F32, tag="nt")
                            nc.scalar.activation(t1[:], pre[:], AF.Square,
                                                 bias=0.0, scale=1.0)
                            t2 = nt.tile([P, CH], F32, tag="nt")
                            nc.scalar.activation(t2[:], pim[:], AF.Square,
                                                 bias=0.0, scale=1.0)
                            m = nt.tile([P, CH], F32, tag="nt")
                            nc.vector.tensor_tensor(m[:], t1[:], t2[:], ALU.add)
                            n = nt.tile([P, CH], F32, tag="nt")
                            nc.scalar.activation(n[:], m[:], AF.Abs_reciprocal_sqrt,
                                                 bias=0.0, scale=1.0)
                            nc.vector.tensor_tensor(oh_re[:, dso, :], pre[:],
                                                    n[:], ALU.mult)
                            nc.vector.tensor_tensor(oh_im[:, dso, :], pim[:],
                                                    n[:], ALU.mult)

                    # final dense partial: Z [tq, D] += Ohat^T.T @ wo_h
                    for ts in range(CH // P):
                        pzre = pools["ps"].tile([P, D], F32, tag="ps")
                        pzim = pools["ps"].tile([P, D], F32, tag="ps")
                        for do in range(DS):
                            nc.tensor.matmul(
                                pzre[:], lhsT=oh_re[:, do, ts * P:(ts + 1) * P],
                                rhs=wo16[:, do, :], start=(do == 0),
                                stop=(do == DS - 1 and h != 0))
                            nc.tensor.matmul(
                                pzim[:], lhsT=oh_im[:, do, ts * P:(ts + 1) * P],
                                rhs=wo16[:, do, :], start=(do == 0),
                                stop=(do == DS - 1))
                        if h == 0:
                            nc.tensor.matmul(
                                pzre[:], lhsT=quart16[:], rhs=bo16[:],
                                start=False, stop=True)
                        trow = ch * (CH // P) + ts
                        if h == 0:
                            nc.scalar.copy(z_re[:, trow, :], pzre[:])
                            nc.scalar.copy(z_im[:, trow, :], pzim[:])
                        else:
                            nc.vector.tensor_tensor(z_re[:, trow, :],
                                                    z_re[:, trow, :], pzre[:],
                                                    ALU.add)
                            nc.vector.tensor_tensor(z_im[:, trow, :],
                                                    z_im[:, trow, :], pzim[:],
                                                    ALU.add)
                            # drain to the collective staging layout:
                            # zb row (per tq quarter qq of 256 rows):
                            #   r*128 + c*64 + i, r = rank-block, c = 0 re/1 im
                            tq0 = ch * CH + ts * P
                            qq = tq0 // 256
                            r0 = 2 * ((tq0 % 256) // P)  # 0 or 2
                            for half in range(2):
                                r_ = r0 + half
                                src = slice(half * 64, (half + 1) * 64)
                                nc.sync.dma_start(
                                    zbs[qq][r_ * P: r_ * P + 64, :],
                                    z_re[src, trow, :])
                                nc.sync.dma_start(
                                    zbs[qq][r_ * P + 64: r_ * P + 128, :],
                                    z_im[src, trow, :])
                            # fire the sub-ReduceScatter once the quarter's
                            # second tile has landed
                            if ts % 2 == 1:
                                nc.gpsimd.collective_compute(
                                    "ReduceScatter", ALU.add,
                                    replica_groups=[[0, 1, 2, 3], [4, 5, 6, 7]],
                                    ins=[zbs[qq].opt()],
                                    outs=[rs_outs[qq].opt()],
                                )

            # ======== atan2(zim, zre)/pi, quarters batched in pairs ========
            for pp in range(2):
                qa, qb = 2 * pp, 2 * pp + 1
                zre_t = pools["nt"].tile([P, D], F32, tag="nt", name=f"zre{pp}")
                nc.sync.dma_start(zre_t[0:64, :], rs_outs[qa][0:64, :])
                nc.sync.dma_start(zre_t[64:128, :], rs_outs[qb][0:64, :])
                zim_t = pools["nt"].tile([P, D], F32, tag="nt", name=f"zim{pp}")
                nc.sync.dma_start(zim_t[0:64, :], rs_outs[qa][64:128, :])
                nc.sync.dma_start(zim_t[64:128, :], rs_outs[qb][64:128, :])
                zre, zim = zre_t[:, :], zim_t[:, :]
                nt = pools["nt"]

                def ft(nm, pp=pp):
                    return nt.tile([P, D], F32, tag="nt", name=f"{nm}{pp}")
                t1 = ft("f1")
                nc.scalar.activation(t1[:], zre, AF.Square, bias=0.0, scale=1.0)
                t2 = ft("f2")
                nc.vector.tensor_tensor(t2[:], zim, zim, ALU.mult)
                m = ft("f3")
                nc.vector.tensor_tensor(m[:], t1[:], t2[:], ALU.add)
                az = ft("f5")
                nc.scalar.activation(az[:], m[:], AF.Sqrt, bias=0.0, scale=1.0)
                den1 = ft("f6")
                nc.vector.tensor_tensor(den1[:], az[:], zre, ALU.add)
                r1 = ft("f7")
                nc.vector.reciprocal_approx_fast(r1[:], den1[:])
                ta0 = ft("f8")
                nc.vector.tensor_tensor(ta0[:], zim, r1[:], ALU.mult)
                ta = ft("f9")
                nc.vector.tensor_scalar(ta[:], ta0[:], 1e8, -1e8, ALU.min, ALU.max)
                num2 = ft("fa")
                nc.vector.tensor_tensor(num2[:], az[:], zre, ALU.subtract)
                r2 = ft("fb")
                nc.vector.reciprocal_approx_fast(r2[:], zim)
                tb0 = ft("fc")
                nc.vector.tensor_tensor(tb0[:], num2[:], r2[:], ALU.mult)
                tb = ft("fd")
                nc.vector.tensor_scalar(tb[:], tb0[:], 1e8, -1e8, ALU.min, ALU.max)
                ata = ft("fe")
                nc.scalar.activation(ata[:], ta[:], AF.Arctan, bias=0.0, scale=1.0)
                atb = ft("ff")
                nc.scalar.activation(atb[:], tb[:], AF.Arctan, bias=0.0, scale=1.0)
                mask = ft("fg")
                nc.vector.tensor_scalar(mask[:], zre, 0.0, None, ALU.is_ge)
                dsel = ft("fh")
                nc.vector.tensor_tensor(dsel[:], ata[:], atb[:], ALU.subtract)
                md = ft("fi")
                nc.vector.tensor_tensor(md[:], mask[:], dsel[:], ALU.mult)
                sel = ft("fj")
                nc.vector.tensor_tensor(sel[:], atb[:], md[:], ALU.add)
                outt = ft("fk")
                nc.vector.tensor_scalar(outt[:], sel[:], 2.0 / PI, None, ALU.mult)
                nc.sync.dma_start(OUT[qa * 64:(qa + 1) * 64, :], outt[0:64, :])
                nc.sync.dma_start(OUT[qb * 64:(qb + 1) * 64, :], outt[64:128, :])

    nc.finalize()
    return nc


_NC_CACHE = {}


def _get_nc():
    if "nc" not in _NC_CACHE:
        _NC_CACHE["nc"] = build()
    return _NC_CACHE["nc"]


def kernel(**inputs):
    query = np.ascontiguousarray(np.asarray(inputs["query"], dtype=np.float32))
    keyvalue = np.ascontiguousarray(np.asarray(inputs["keyvalue"], dtype=np.float32))
    wq = np.asarray(inputs["wq"], dtype=np.float32)
    wk = np.asarray(inputs["wk"], dtype=np.float32)
    wv = np.asarray(inputs["wv"], dtype=np.float32)
    bq = np.asarray(inputs["bq"], dtype=np.float32)
    bk = np.asarray(inputs["bk"], dtype=np.float32)
    bv = np.asarray(inputs["bv"], dtype=np.float32)
    wo = np.asarray(inputs["wo"], dtype=np.float32)
    bo = np.asarray(inputs["bo"], dtype=np.float32)

    in_maps = []
    for c in range(N_CORES):
        b, g = c // 4, c % 4
        h0 = g * HPC
        in_maps.append({
            "query": query[b],
            "keyvalue": keyvalue[b],
            "wq": np.ascontiguousarray(wq[h0:h0 + HPC]),
            "wk": np.ascontiguousarray(wk[h0:h0 + HPC]),
            "wv": np.ascontiguousarray(wv[h0:h0 + HPC]),
            "bq": np.ascontiguousarray(bq[h0:h0 + HPC]),
            "bk": np.ascontiguousarray(bk[h0:h0 + HPC]),
            "bv": np.ascontiguousarray(bv[h0:h0 + HPC]),
            "wo": np.ascontiguousarray(wo[h0 * D:(h0 + HPC) * D]),
            "bo": bo,
        })

    nc = _get_nc()
    res = run_bass_kernel_spmd(nc, in_maps, core_ids=list(range(N_CORES)))
    _NC_CACHE["last_results"] = res
    out = np.empty((B, T, D), np.float32)
    for c in range(N_CORES):
        b, g = c // 4, c % 4
        o = res.results[c]["out"]          # [256, 512]: 4 quarters x 64 rows
        for qq in range(4):
            out[b, qq * 256 + g * 64: qq * 256 + (g + 1) * 64, :] = \
                o[qq * 64:(qq + 1) * 64, :]
    return out
